# revision 24
# baseline (speedup 1.0000x reference)
"""BottleneckMamba Trainium2 kernel (self-contained).

out = x + cv2( scale * out_proj( LN(cross-merge(4-dir selective scan(N=1))) * z ) )

3 SPMD launches on 8 NeuronCores:
  L1 (core=(b, image-half)): cv1 -> h; depthwise3x3*in_proj folded into 9
     matmuls (8-row strips, double-buffered PSUM) -> silu -> xc ;
     z = silu(Wz@h) ; B/C rows via PE; ub_k = xc*B_k prefolded on DVE.
  L2 (core=(b, dir-group)): per dir: dtd matmul -> exp/ln1p/exp on ACT
     (one table set) -> b = dt*ub -> tensor_tensor_scan (reversed-AP for the
     backward dir) -> h*C ; PE merges the pair + D*u -> m.
  L3 (core=(b, half)): per-chunk pipeline: y = m02 + m13^T, LN stats via
     ones-matmuls + row pipeline, DMA-replicated LN rows, fused
     (cv2 @ diag(scale) @ out_proj) matmul, bf16 output.
Host: shards/reassembles, transposes between launches, adds residual x.
"""
import os
import sys

sys.path.insert(0, '/opt/trn_rl_repo')

import numpy as np
import ml_dtypes

import concourse.bass as bass
import concourse.tile as tile
import concourse.mybir as mybir
from concourse.bass_utils import run_bass_kernel_spmd

bf16 = mybir.dt.bfloat16
f32 = mybir.dt.float32
MULT, ADD = mybir.AluOpType.mult, mybir.AluOpType.add
SUB = mybir.AluOpType.subtract
AF = mybir.ActivationFunctionType
NBF = ml_dtypes.bfloat16

B, C1, C2, H, W = 4, 256, 256, 128, 128
Cm, K, R = 128, 4, 8
L = H * W          # 16384
HH = H // 2        # 64 rows per half
LH = HH * W        # 8192
CH = 2048          # L2/L3 chunk
NCH = L // CH      # 8

EXEC_TIMES = {}    # launch -> exec ns (MAMBA_TRACE=1)
_CACHE = {}


def _split_multiwaits(nc):
    """walrus here accepts ONE sync-wait per instruction; hoist extras into
    single-wait same-engine NOPs inserted before the instruction."""
    for f in nc.m.functions:
        for bb in f.blocks:
            il = bb.instructions
            i = 0
            while i < len(il):
                ins = il[i]
                si = getattr(ins, "sync_info", None)
                if si is not None and len(si.on_wait) > 1:
                    waits = list(si.on_wait)
                    ins.sync_info = mybir.SyncInfo(
                        on_wait=[waits[-1]], on_update=list(si.on_update))
                    for w in waits[:-1]:
                        nop = mybir.InstNoOp(
                            name=nc.get_next_instruction_name(), ins=[], outs=[])
                        nop.engine = ins.engine
                        nop.sync_info = mybir.SyncInfo(on_wait=[w], on_update=[])
                        nc.register_instruction(nop, overwrite=True)
                        il.insert(i, nop)
                        i += 1
                i += 1


def _new_nc():
    return bass.Bass("TRN2", target_bir_lowering=False, debug=False,
                     enable_asserts=True, num_devices=8)


def _run(nc, in_maps, name):
    trace = os.environ.get("MAMBA_TRACE", "0") == "1"
    res = run_bass_kernel_spmd(nc, in_maps, core_ids=list(range(8)), trace=trace)
    if trace:
        EXEC_TIMES[name] = res.exec_time_ns
    return res.results


def _rep_ap(t, row, sl):
    """DRAM [rows, width] tile row slice -> partition-replicated AP."""
    width = t.shape[-1]
    return bass.AP(tensor=t.tensor if hasattr(t, 'tensor') else t,
                   offset=row * width + sl.start,
                   ap=[[0, 128], [1, sl.stop - sl.start]])


def _mm(nc, reuse_w, **kw):
    """matmul; reuse_w=True skips the implicit LDWEIGHTS (same lhsT as the
    immediately preceding PE matmul)."""
    h = nc.tensor.matmul(**kw)
    if reuse_w:
        h.ins.ldweights = False
    return h


# ------------------------------------------------------------------- L1
def build_l1():
    nc = _new_nc()
    x_in = nc.dram_tensor("x_in", [C1, HH + 2, W], bf16, kind="ExternalInput")
    wcv1 = nc.dram_tensor("wcv1", [C1, Cm], f32, kind="ExternalInput")       # lhsT
    bcv1 = nc.dram_tensor("bcv1", [Cm, 1], f32, kind="ExternalInput")
    wfold = nc.dram_tensor("wfold", [Cm, 9, Cm], f32, kind="ExternalInput")  # (k, tap, m)
    bconv = nc.dram_tensor("bconv", [Cm, 1], f32, kind="ExternalInput")
    wz = nc.dram_tensor("wz", [Cm, Cm], f32, kind="ExternalInput")           # lhsT
    wbc = nc.dram_tensor("wbc", [Cm, 8], f32, kind="ExternalInput")          # lhsT
    hmask = nc.dram_tensor("hmask", [Cm, 2], f32, kind="ExternalInput")
    wdt = nc.dram_tensor("wdt", [4, Cm, Cm], f32, kind="ExternalInput")   # lhsT per dir
    dtb = nc.dram_tensor("dtb", [Cm, 4], f32, kind="ExternalInput")
    xc_out = nc.dram_tensor("xc_out", [Cm, LH], bf16, kind="ExternalOutput")
    z_out = nc.dram_tensor("z_out", [Cm, LH], bf16, kind="ExternalOutput")
    bcr_out = nc.dram_tensor("bcr_out", [8, LH], bf16, kind="ExternalOutput")
    dt_out = nc.dram_tensor("dt_out", [4, Cm, LH], bf16, kind="ExternalOutput")
    bt_out = nc.dram_tensor("bt_out", [4, Cm, LH], bf16, kind="ExternalOutput")

    HP = HH + 2   # 66
    WP = W + 2    # 130
    SR = 8        # strip rows
    SC = SR * W   # 1024
    brow = {0: 0, 2: 2, 1: 4, 3: 6}   # dir -> B row in bcr stack

    with tile.TileContext(nc) as tc, \
         tc.tile_pool(name="w", bufs=1) as wp, \
         tc.tile_pool(name="d", bufs=1) as dp, \
         tc.tile_pool(name="br", bufs=3) as brp, \
         tc.tile_pool(name="dr", bufs=1, space="DRAM") as drp, \
         tc.tile_pool(name="psA", bufs=2, space="PSUM") as ppA, \
         tc.tile_pool(name="psB", bufs=2, space="PSUM") as ppB:
        tw1a = wp.tile([128, Cm], bf16)
        tw1b = wp.tile([128, Cm], bf16)
        nc.gpsimd.dma_start(out=tw1a, in_=wcv1[0:128, :])
        nc.gpsimd.dma_start(out=tw1b, in_=wcv1[128:256, :])
        twf = wp.tile([Cm, 9, Cm], bf16)
        nc.gpsimd.dma_start(out=twf, in_=wfold[:, :, :])
        twz = wp.tile([Cm, Cm], bf16)
        nc.gpsimd.dma_start(out=twz, in_=wz[:, :])
        twbc = wp.tile([Cm, 8], bf16)
        nc.gpsimd.dma_start(out=twbc, in_=wbc[:, :])
        tb1 = wp.tile([Cm, 1], f32)
        nc.sync.dma_start(out=tb1, in_=bcv1[:, :])
        tbc = wp.tile([Cm, 1], f32)
        nc.sync.dma_start(out=tbc, in_=bconv[:, :])
        tmask = wp.tile([Cm, 2], f32)
        nc.sync.dma_start(out=tmask, in_=hmask[:, :])
        twdt = wp.tile([Cm, 4, Cm], bf16)
        nc.gpsimd.dma_start(out=twdt, in_=wdt[:, :, :])
        tdtb = wp.tile([Cm, 4], f32)
        nc.sync.dma_start(out=tdtb, in_=dtb[:, :])

        txa = dp.tile([128, HP, W], bf16)
        txb = dp.tile([128, HP, W], bf16)
        for rb in range(0, HP, 11):
            nc.gpsimd.dma_start(out=txa[:, rb:rb + 11, :], in_=x_in[0:128, rb:rb + 11, :])
            nc.gpsimd.dma_start(out=txb[:, rb:rb + 11, :], in_=x_in[128:256, rb:rb + 11, :])

        th = dp.tile([Cm, HP, WP], bf16)
        nc.vector.memset(th[:, :, 0:1], 0.0)
        nc.vector.memset(th[:, :, WP - 1:WP], 0.0)

        # cv1 over 66 rows: 8 strips of 8 + 1 strip of 2 (DVE does psum copy)
        row_chunks = [(r0, SR) for r0 in range(0, 64, SR)] + [(64, 2)]
        for r0, nr in row_chunks:
            pt = ppA.tile([Cm, SC], f32, tag="cv1")
            nn = nr * W
            for j in range(0, nn, 512):
                je = min(j + 512, nn)
                ra, rb_ = r0 + j // W, r0 + je // W
                _mm(nc, j > 0, out=pt[:, j:je], lhsT=tw1a[:, :],
                    rhs=txa[:, ra:rb_, :], start=True, stop=False)
            for j in range(0, nn, 512):
                je = min(j + 512, nn)
                ra, rb_ = r0 + j // W, r0 + je // W
                _mm(nc, j > 0, out=pt[:, j:je], lhsT=tw1b[:, :],
                    rhs=txb[:, ra:rb_, :], start=False, stop=True)
            nc.vector.tensor_scalar(out=th[:, r0:r0 + nr, 1:W + 1],
                                    in0=pt[:, :nn], scalar1=tb1[:, :],
                                    scalar2=None, op0=ADD)
        nc.vector.tensor_scalar_mul(out=th[:, 0, :], in0=th[:, 0, :],
                                    scalar1=tmask[:, 0:1])
        nc.vector.tensor_scalar_mul(out=th[:, HP - 1, :], in0=th[:, HP - 1, :],
                                    scalar1=tmask[:, 1:2])

        txc = dp.tile([Cm, LH], bf16)
        tz = dp.tile([Cm, LH], bf16)
        tbcr = dp.tile([8, LH], bf16)
        bcrt = [drp.tile([8, SC], bf16, name=f"bcrt{i}") for i in range(HH // SR)]

        tubf = [dp.tile([Cm, LH], bf16, name=f"tubf{k}") for k in range(4)]

        def ub_work(rs):
            slu = slice(rs * W, (rs + SR) * W)
            for k in range(4):
                tbr = brp.tile([Cm, SC], bf16, tag="brep")
                nc.scalar.dma_start(
                    out=tbr, in_=_rep_ap(bcrt[rs // SR], brow[k], slice(0, SC)))
                nc.vector.tensor_tensor(out=tubf[k][:, slu], in0=txc[:, slu],
                                        in1=tbr, op=MULT)
        for r0 in range(0, HH, SR):
            sl = slice(r0 * W, (r0 + SR) * W)
            pt = ppB.tile([Cm, SC], f32, tag="fold")
            for t in range(9):
                dy, dx = t // 3 - 1, t % 3 - 1
                for j in range(2):
                    rj = r0 + 1 + dy + 4 * j
                    _mm(nc, j > 0, out=pt[:, j * 512:(j + 1) * 512],
                        lhsT=twf[:, t, :],
                        rhs=th[:, rj:rj + 4, 1 + dx:W + 1 + dx],
                        start=(t == 0), stop=(t == 8))
            nc.scalar.activation(out=txc[:, sl], in_=pt[:, :],
                                 func=AF.Silu, bias=tbc[:, :], scale=1.0)
            ptz = ppA.tile([Cm, SC], f32, tag="cv1")
            for j in range(2):
                rj = r0 + 1 + 4 * j
                _mm(nc, j > 0, out=ptz[:, j * 512:(j + 1) * 512], lhsT=twz[:, :],
                    rhs=th[:, rj:rj + 4, 1:W + 1], start=True, stop=True)
            nc.scalar.activation(out=tz[:, sl], in_=ptz[:, :],
                                 func=AF.Silu, bias=0.0, scale=1.0)
            ptb = ppA.tile([Cm, SC], f32, tag="cv1")
            for j in range(2):
                s5 = slice(r0 * W + j * 512, r0 * W + (j + 1) * 512)
                _mm(nc, j > 0, out=ptb[0:8, j * 512:(j + 1) * 512], lhsT=twbc[:, :],
                    rhs=txc[:, s5], start=True, stop=True)
            nc.vector.tensor_copy(out=tbcr[:, sl], in_=ptb[0:8, :])
            nc.gpsimd.dma_start(out=bcrt[r0 // SR][:, :], in_=tbcr[:, sl])
            nc.gpsimd.dma_start(out=xc_out[:, sl], in_=txc[:, sl])
            nc.gpsimd.dma_start(out=z_out[:, sl], in_=tz[:, sl])
            # ub for strip r0-2*SR (2-strip lag hides the DRAM roundtrip)
            if r0 >= 2 * SR:
                ub_work(r0 - 2 * SR)
        ub_work(HH - 2 * SR)
        ub_work(HH - SR)
        nc.sync.dma_start(out=bcr_out[:, :], in_=tbcr[:, :])
        # pass B: dtd matmul -> e1 -> dt (ln1p) -> bt = dt*ub, all 4 dirs
        for r0 in range(0, HH, SR):
            sl = slice(r0 * W, (r0 + SR) * W)
            for k in range(4):
                pt = ppB.tile([Cm, SC], f32, tag="fold")
                for j in range(2):
                    s5 = slice(r0 * W + j * 512, r0 * W + (j + 1) * 512)
                    nc.tensor.matmul(out=pt[:, j * 512:(j + 1) * 512],
                                     lhsT=twdt[:, k, :], rhs=txc[:, s5],
                                     start=True, stop=True)
                te1 = brp.tile([Cm, SC], bf16, tag="e1")
                nc.scalar.activation(out=te1, in_=pt[:, :], func=AF.Exp,
                                     bias=tdtb[:, k:k + 1], scale=1.0)
                tdt = brp.tile([Cm, SC], bf16, tag="dt")
                nc.scalar.activation(out=tdt, in_=te1, func=AF.Ln,
                                     bias=1.0, scale=1.0)
                nc.sync.dma_start(out=dt_out[k, :, sl], in_=tdt)
                tbt = brp.tile([Cm, SC], bf16, tag="bt")
                nc.vector.tensor_tensor(out=tbt, in0=tdt, in1=tubf[k][:, sl],
                                        op=MULT)
                nc.gpsimd.dma_start(out=bt_out[k, :, sl], in_=tbt)
    return nc


# ------------------------------------------------------------------- L2
def build_l2():
    nc = _new_nc()
    u_in = nc.dram_tensor("u_in", [Cm, L], bf16, kind="ExternalInput")
    dt_f = nc.dram_tensor("dt_f", [Cm, L], bf16, kind="ExternalInput")
    dt_r = nc.dram_tensor("dt_r", [Cm, L], bf16, kind="ExternalInput")
    bt_f = nc.dram_tensor("bt_f", [Cm, L], bf16, kind="ExternalInput")
    bt_r = nc.dram_tensor("bt_r", [Cm, L], bf16, kind="ExternalInput")
    a_f = nc.dram_tensor("a_f", [Cm, 1], f32, kind="ExternalInput")
    a_r = nc.dram_tensor("a_r", [Cm, 1], f32, kind="ExternalInput")
    crow_f = nc.dram_tensor("crow_f", [1, L], bf16, kind="ExternalInput")
    crow_r = nc.dram_tensor("crow_r", [1, L], bf16, kind="ExternalInput")
    ident = nc.dram_tensor("ident", [Cm, Cm], f32, kind="ExternalInput")
    diagd = nc.dram_tensor("diagd", [Cm, Cm], f32, kind="ExternalInput")
    m_out = nc.dram_tensor("m_out", [Cm, L], bf16, kind="ExternalOutput")

    def bc_ap(t, sl):  # DRAM row slice -> partition-replicated AP
        return bass.AP(tensor=t, offset=sl.start, ap=[[0, 128], [1, sl.stop - sl.start]])

    with tile.TileContext(nc) as tc, \
         tc.tile_pool(name="w", bufs=1) as wp, \
         tc.tile_pool(name="u", bufs=1) as up, \
         tc.tile_pool(name="full", bufs=1) as fp, \
         tc.tile_pool(name="ck", bufs=2) as cp, \
         tc.tile_pool(name="hk", bufs=2) as hp, \
         tc.tile_pool(name="bc", bufs=3) as bp, \
         tc.tile_pool(name="psm", bufs=2, space="PSUM") as psm:
        taf = wp.tile([Cm, 1], f32)
        tar = wp.tile([Cm, 1], f32)
        nc.sync.dma_start(out=taf, in_=a_f[:, :])
        nc.sync.dma_start(out=tar, in_=a_r[:, :])
        tid = wp.tile([Cm, Cm], bf16)
        tdg = wp.tile([Cm, Cm], bf16)
        nc.gpsimd.dma_start(out=tid, in_=ident[:, :])
        nc.gpsimd.dma_start(out=tdg, in_=diagd[:, :])

        tu = up.tile([Cm, L], bf16)
        uorder = []
        for s in range(NCH // 2):
            uorder += [NCH - 1 - s, s]
        for ci in uorder:
            nc.sync.dma_start(out=tu[:, ci * CH:(ci + 1) * CH],
                              in_=u_in[:, ci * CH:(ci + 1) * CH])

        tmcr = fp.tile([Cm, L], bf16)   # h_r * C_r, natural position order

        def chunk_front(ci, ta, dtt, btt):
            """load dt/bt, a = exp(A*dt) for chunk ci."""
            sl = slice(ci * CH, (ci + 1) * CH)
            tdt = cp.tile([Cm, CH], bf16, tag="dt")
            nc.gpsimd.dma_start(out=tdt, in_=dtt[:, sl])
            tav = cp.tile([Cm, CH], bf16, tag="av")
            nc.scalar.activation(out=tav, in_=tdt, func=AF.Exp,
                                 bias=0.0, scale=ta[:, :])
            tbt = cp.tile([Cm, CH], bf16, tag="bt")
            nc.scalar.dma_start(out=tbt, in_=btt[:, sl])
            return tav, tbt

        tmcf = fp.tile([Cm, L], bf16)   # h_f * C_f

        def do_merge(ci):
            for q in range(CH // 1024):
                pm = psm.tile([Cm, 1024], f32, tag="mp")
                base = ci * CH + q * 1024
                for v in range(2):
                    nc.tensor.matmul(
                        out=pm[:, v * 512:(v + 1) * 512], lhsT=tid[:, :],
                        rhs=tmcf[:, base + v * 512:base + (v + 1) * 512],
                        start=True, stop=False)
                for v in range(2):
                    nc.tensor.matmul(
                        out=pm[:, v * 512:(v + 1) * 512], lhsT=tid[:, :],
                        rhs=tmcr[:, base + v * 512:base + (v + 1) * 512],
                        start=False, stop=False)
                for v in range(2):
                    nc.tensor.matmul(
                        out=pm[:, v * 512:(v + 1) * 512], lhsT=tdg[:, :],
                        rhs=tu[:, base + v * 512:base + (v + 1) * 512],
                        start=False, stop=True)
                tm = hp.tile([Cm, 1024], bf16, tag="m")
                if q == 0:
                    nc.scalar.activation(out=tm, in_=pm[:, :],
                                         func=AF.Identity, bias=0.0, scale=1.0)
                else:
                    nc.vector.tensor_copy(out=tm, in_=pm[:, :])
                nc.gpsimd.dma_start(out=m_out[:, base:base + 1024], in_=tm)

        # ---- interleaved rev (descending) + fwd (ascending) scans ----
        prev_r = None
        prev_f = None
        for s in range(NCH):
            cr = NCH - 1 - s
            cf = s
            # reverse-direction chunk
            tav, tbt = chunk_front(cr, tar, dt_r, bt_r)
            thc = hp.tile([Cm, CH], bf16, tag="hr")
            nc.vector.tensor_tensor_scan(
                out=thc[:, ::-1], data0=tav[:, ::-1], data1=tbt[:, ::-1],
                initial=0.0 if prev_r is None else prev_r, op0=MULT, op1=ADD)
            prev_r = thc[:, 0:1]
            sl = slice(cr * CH, (cr + 1) * CH)
            tcc = bp.tile([Cm, CH], bf16, tag="cbc")
            nc.gpsimd.dma_start(out=tcc, in_=bc_ap(crow_r, sl))
            nc.vector.tensor_tensor(out=tmcr[:, sl], in0=thc, in1=tcc, op=MULT)
            # forward-direction chunk
            tav, tbt = chunk_front(cf, taf, dt_f, bt_f)
            thcf = hp.tile([Cm, CH], bf16, tag="hf")
            nc.vector.tensor_tensor_scan(
                out=thcf, data0=tav, data1=tbt,
                initial=0.0 if prev_f is None else prev_f, op0=MULT, op1=ADD)
            prev_f = thcf[:, CH - 1:CH]
            slf = slice(cf * CH, (cf + 1) * CH)
            tcf = bp.tile([Cm, CH], bf16, tag="cbc")
            nc.gpsimd.dma_start(out=tcf, in_=bc_ap(crow_f, slf))
            nc.vector.tensor_tensor(out=tmcf[:, slf], in0=thcf, in1=tcf, op=MULT)
            # merges whose inputs are now complete
            if s >= NCH // 2:
                do_merge(NCH - 1 - s)
                do_merge(s)
    return nc


# ------------------------------------------------------------------- L3
def build_l3():
    nc = _new_nc()
    m02 = nc.dram_tensor("m02", [Cm, LH], bf16, kind="ExternalInput")
    m13 = nc.dram_tensor("m13", [Cm, LH], bf16, kind="ExternalInput")
    z_in = nc.dram_tensor("z_in", [Cm, LH], bf16, kind="ExternalInput")
    lng = nc.dram_tensor("lng", [Cm, 1], f32, kind="ExternalInput")
    lnb = nc.dram_tensor("lnb", [Cm, 1], f32, kind="ExternalInput")
    wfin = nc.dram_tensor("wfin", [Cm, C2], f32, kind="ExternalInput")
    bfin = nc.dram_tensor("bfin", [128, 2], f32, kind="ExternalInput")
    ones128 = nc.dram_tensor("ones128", [Cm, 1], f32, kind="ExternalInput")
    d_out = nc.dram_tensor("d_out", [C2, LH], bf16, kind="ExternalOutput")

    NC3 = LH // CH   # 4
    QC = CH // 128   # 16  (row-pipeline cols per chunk)

    with tile.TileContext(nc) as tc, \
         tc.tile_pool(name="w", bufs=1) as wp, \
         tc.tile_pool(name="d", bufs=1) as dp, \
         tc.tile_pool(name="c", bufs=2) as cp, \
         tc.tile_pool(name="dr", bufs=1, space="DRAM") as drp, \
         tc.tile_pool(name="ps1", bufs=1, space="PSUM") as ps1, \
         tc.tile_pool(name="ps2", bufs=1, space="PSUM") as ps2:
        tg = wp.tile([Cm, 1], f32)
        tb = wp.tile([Cm, 1], f32)
        nc.sync.dma_start(out=tg, in_=lng[:, :])
        nc.sync.dma_start(out=tb, in_=lnb[:, :])
        twa = wp.tile([Cm, 128], bf16)
        twb = wp.tile([Cm, 128], bf16)
        nc.gpsimd.dma_start(out=twa, in_=wfin[:, 0:128])
        nc.gpsimd.dma_start(out=twb, in_=wfin[:, 128:256])
        tbf = wp.tile([128, 2], f32)
        nc.sync.dma_start(out=tbf, in_=bfin[:, :])
        tone = wp.tile([Cm, 1], bf16)
        nc.gpsimd.dma_start(out=tone, in_=ones128[:, :])
        teps = wp.tile([128, 1], f32)
        nc.vector.memset(teps, 1e-5)

        tm0 = dp.tile([Cm, LH], bf16)
        tm1 = dp.tile([Cm, LH], bf16)
        tz = dp.tile([Cm, LH], bf16)
        ty = dp.tile([Cm, LH], bf16)
        tmu = dp.tile([1, LH], bf16)
        tss = dp.tile([1, LH], bf16)
        rowt = [drp.tile([2, CH], bf16, name=f"rowt{i}") for i in range(NC3)]

        for ci in range(NC3):
            sl = slice(ci * CH, (ci + 1) * CH)
            nc.sync.dma_start(out=tm0[:, sl], in_=m02[:, sl])
            nc.sync.dma_start(out=tm1[:, sl], in_=m13[:, sl])
            nc.sync.dma_start(out=tz[:, sl], in_=z_in[:, sl])
            nc.vector.tensor_tensor(out=ty[:, sl], in0=tm0[:, sl],
                                    in1=tm1[:, sl], op=ADD)
            tsq = cp.tile([Cm, CH], bf16, tag="sq")
            nc.vector.tensor_tensor(out=tsq, in0=ty[:, sl], in1=ty[:, sl], op=MULT)
            # stats (means via 1/Cm-ones lhsT)
            for q in range(2):
                pmu = ps2.tile([1, 1024], f32, tag="smu")
                pss = ps2.tile([1, 1024], f32, tag="sss")
                for v in range(2):
                    pv = slice(v * 512, (v + 1) * 512)
                    gv = slice(ci * CH + q * 1024 + v * 512,
                               ci * CH + q * 1024 + (v + 1) * 512)
                    lv = slice(q * 1024 + v * 512, q * 1024 + (v + 1) * 512)
                    _mm(nc, not (ci == 0 and q == 0 and v == 0),
                        out=pmu[:, pv], lhsT=tone[:, :],
                        rhs=ty[:, gv], start=True, stop=True)
                    _mm(nc, True, out=pss[:, pv], lhsT=tone[:, :],
                        rhs=tsq[:, lv], start=True, stop=True)
                sog = slice(ci * CH + q * 1024, ci * CH + (q + 1) * 1024)
                nc.scalar.activation(out=tmu[:, sog], in_=pmu[:, :],
                                     func=AF.Identity, bias=0.0, scale=1.0)
                nc.scalar.activation(out=tss[:, sog], in_=pss[:, :],
                                     func=AF.Identity, bias=0.0, scale=1.0)
            # row pipeline for this chunk: [128, 16]
            tmu2 = cp.tile([128, QC], f32, tag="r1")
            tss2 = cp.tile([128, QC], f32, tag="r2")
            nc.gpsimd.dma_start(out=tmu2[:, :], in_=tmu[0:1, sl])
            nc.gpsimd.dma_start(out=tss2[:, :], in_=tss[0:1, sl])
            tvar = cp.tile([128, QC], f32, tag="r3")
            nc.vector.tensor_tensor(out=tvar, in0=tmu2, in1=tmu2, op=MULT)
            nc.vector.tensor_tensor(out=tvar, in0=tss2, in1=tvar, op=SUB)
            tlnv = cp.tile([128, QC], f32, tag="r4")
            nc.scalar.activation(out=tlnv, in_=tvar, func=AF.Ln,
                                 bias=teps[:, :], scale=1.0)
            trst = cp.tile([128, QC], bf16, tag="r5")
            nc.scalar.activation(out=trst, in_=tlnv, func=AF.Exp,
                                 bias=0.0, scale=-0.5)
            tmr = cp.tile([128, QC], bf16, tag="r6")
            nc.vector.scalar_tensor_tensor(out=tmr, in0=tmu2, scalar=-1.0,
                                           in1=trst, op0=MULT, op1=MULT)
            nc.sync.dma_start(out=rowt[ci][0:1, :], in_=trst[:, :])
            nc.sync.dma_start(out=rowt[ci][1:2, :], in_=tmr[:, :])

        for ci in range(NC3):
            sl = slice(ci * CH, (ci + 1) * CH)
            # apply
            tR = cp.tile([Cm, CH], bf16, tag="Rr")
            tS = cp.tile([Cm, CH], bf16, tag="Sr")
            nc.gpsimd.dma_start(out=tR, in_=_rep_ap(rowt[ci], 0, slice(0, CH)))
            nc.gpsimd.dma_start(out=tS, in_=_rep_ap(rowt[ci], 1, slice(0, CH)))
            tt = cp.tile([Cm, CH], bf16, tag="t")
            nc.vector.tensor_tensor(out=tt, in0=ty[:, sl], in1=tR, op=MULT)
            nc.vector.tensor_tensor(out=tt, in0=tt, in1=tS, op=ADD)
            nc.vector.tensor_scalar(out=tt, in0=tt, scalar1=tg[:, :],
                                    scalar2=tb[:, :], op0=MULT, op1=ADD)
            nc.vector.tensor_tensor(out=tt, in0=tt, in1=tz[:, sl], op=MULT)
            # final matmul per 1024
            for q in range(CH // 1024):
                pda = ps1.tile([128, 1024], f32, tag="da")
                pdb = ps1.tile([128, 1024], f32, tag="db")
                for v in range(2):
                    pv = slice(v * 512, (v + 1) * 512)
                    lv = slice(q * 1024 + v * 512, q * 1024 + (v + 1) * 512)
                    _mm(nc, v > 0, out=pda[:, pv], lhsT=twa[:, :], rhs=tt[:, lv],
                        start=True, stop=True)
                for v in range(2):
                    pv = slice(v * 512, (v + 1) * 512)
                    lv = slice(q * 1024 + v * 512, q * 1024 + (v + 1) * 512)
                    _mm(nc, v > 0, out=pdb[:, pv], lhsT=twb[:, :], rhs=tt[:, lv],
                        start=True, stop=True)
                sog = slice(ci * CH + q * 1024, ci * CH + (q + 1) * 1024)
                tda = cp.tile([128, 1024], bf16, tag="oa")
                tdb = cp.tile([128, 1024], bf16, tag="ob")
                nc.scalar.activation(out=tda, in_=pda[:, :], func=AF.Identity,
                                     bias=tbf[:, 0:1], scale=1.0)
                nc.scalar.activation(out=tdb, in_=pdb[:, :], func=AF.Identity,
                                     bias=tbf[:, 1:2], scale=1.0)
                nc.sync.dma_start(out=d_out[0:128, sog], in_=tda)
                nc.gpsimd.dma_start(out=d_out[128:256, sog], in_=tdb)
    return nc


# ------------------------------------------------------------------- host
def _get_ncs():
    if "ncs" not in _CACHE:
        nc1, nc2, nc3 = build_l1(), build_l2(), build_l3()
        for n in (nc1, nc2, nc3):
            _split_multiwaits(n)
        _CACHE["ncs"] = (nc1, nc2, nc3)
    return _CACHE["ncs"]


def kernel(x, cv1_w, cv1_b, scale_w, in_proj_w, conv_w, conv_b, x_proj_w,
           dt_w, dt_b, A_logs, Ds, ln_g, ln_b, out_proj_w, cv2_w, cv2_b):
    f = np.float32
    x = np.asarray(x, f)
    cv1_w = np.asarray(cv1_w, f); cv1_b = np.asarray(cv1_b, f)
    in_proj_w = np.asarray(in_proj_w, f)
    conv_w = np.asarray(conv_w, f); conv_b = np.asarray(conv_b, f)
    x_proj_w = np.asarray(x_proj_w, f)
    dt_w = np.asarray(dt_w, f); dt_b = np.asarray(dt_b, f)
    A_logs = np.asarray(A_logs, f); Ds = np.asarray(Ds, f)
    ln_g = np.asarray(ln_g, f); ln_b = np.asarray(ln_b, f)
    out_proj_w = np.asarray(out_proj_w, f)
    cv2_w = np.asarray(cv2_w, f); cv2_b = np.asarray(cv2_b, f)
    scale_v = np.asarray(scale_w, f).reshape(Cm)

    Wip_x, Wip_z = in_proj_w[:Cm], in_proj_w[Cm:]
    dwk = conv_w[:, 0]
    A = -np.exp(A_logs).reshape(K, Cm)
    Dk = Ds.reshape(K, Cm)
    W_dtk = np.einsum('kdr,krc->kdc', dt_w, x_proj_w[:, :R])
    WB, WC = x_proj_w[:, R], x_proj_w[:, R + 1]
    W_final = cv2_w @ (scale_v[:, None] * out_proj_w)

    # fold lhsT: (tap, k=h-chan, m=out-chan) -> host layout (k, tap, m)
    Wfold = np.einsum('cyx,cd->yxdc', dwk, Wip_x)      # (3,3, in, out)
    wfold_rm = np.ascontiguousarray(
        Wfold.reshape(9, Cm, Cm).transpose(1, 0, 2))   # row-major cores
    wbc_l = np.stack([WB[0], WC[0], WB[2], WC[2],
                      WB[1], WC[1], WB[3], WC[3]], axis=1)
    wdt_l = np.ascontiguousarray(
        np.stack([W_dtk[k].T for k in range(4)], axis=1))  # (Cm, 4, Cm)

    nc1, nc2, nc3 = _get_ncs()

    # ---------------- L1 ----------------
    l1_maps = []
    for core in range(8):
        b, half = core // 2, core % 2
        r0 = half * HH
        xs = np.zeros((C1, HH + 2, W), NBF)
        lo, hi = r0 - 1, r0 + HH + 1
        slo, shi = max(lo, 0), min(hi, H)
        xs[:, slo - lo: shi - lo, :] = x[b, :, slo:shi, :].astype(NBF)
        mask = np.ones((Cm, 2), np.float32)
        mask[:, 0] = 0.0 if half == 0 else 1.0
        mask[:, 1] = 1.0 if half == 0 else 0.0
        l1_maps.append({
            "x_in": xs,
            "wcv1": np.ascontiguousarray(cv1_w.T),
            "bcv1": cv1_b.reshape(Cm, 1),
            "wfold": wfold_rm,
            "bconv": conv_b.reshape(Cm, 1),
            "wz": np.ascontiguousarray(Wip_z.T),
            "wbc": np.ascontiguousarray(wbc_l),
            "wdt": wdt_l,
            "dtb": np.ascontiguousarray(dt_b.T.reshape(Cm, 4)),
            "hmask": mask,
        })
    r1 = _run(nc1, l1_maps, "L1")

    xc = np.zeros((B, Cm, L), NBF)
    zf = np.zeros((B, Cm, L), NBF)
    rows = np.zeros((B, 8, L), NBF)
    dtv = np.zeros((B, 4, Cm, L), NBF)
    btv = np.zeros((B, 4, Cm, L), NBF)
    for core in range(8):
        b, half = core // 2, core % 2
        sl = slice(half * LH, (half + 1) * LH)
        xc[b][:, sl] = r1[core]["xc_out"]
        zf[b][:, sl] = r1[core]["z_out"]
        rows[b][:, sl] = r1[core]["bcr_out"]
        dtv[b][:, :, sl] = r1[core]["dt_out"]
        btv[b][:, :, sl] = r1[core]["bt_out"]

    # ---------------- L2 ----------------
    def t_spatial(a):
        return np.ascontiguousarray(
            a.reshape(*a.shape[:-1], H, W).swapaxes(-1, -2).reshape(*a.shape[:-1], L))

    ident = np.eye(Cm, dtype=np.float32)
    l2_maps = []
    for core in range(8):
        b, g = core // 2, core % 2
        if g == 0:
            u = xc[b]
            kf, kr = 0, 2
            dtf, dtr = dtv[b][0], dtv[b][2]
            btf, btr = btv[b][0], btv[b][2]
            cr_f, cr_r = rows[b][1], rows[b][3]
        else:
            u = t_spatial(xc[b])
            kf, kr = 1, 3
            dtf, dtr = t_spatial(dtv[b][1]), t_spatial(dtv[b][3])
            btf, btr = t_spatial(btv[b][1]), t_spatial(btv[b][3])
            cr_f, cr_r = t_spatial(rows[b][5]), t_spatial(rows[b][7])
        dsum_v = (Dk[kf] + Dk[kr]).astype(np.float32)
        l2_maps.append({
            "u_in": np.ascontiguousarray(u),
            "dt_f": np.ascontiguousarray(dtf), "dt_r": np.ascontiguousarray(dtr),
            "bt_f": np.ascontiguousarray(btf), "bt_r": np.ascontiguousarray(btr),
            "a_f": A[kf].reshape(Cm, 1).astype(f), "a_r": A[kr].reshape(Cm, 1).astype(f),
            "crow_f": cr_f.reshape(1, L), "crow_r": cr_r.reshape(1, L),
            "ident": ident, "diagd": np.diag(dsum_v).astype(np.float32),
        })
    r2 = _run(nc2, l2_maps, "L2")

    # ---------------- L3 ----------------
    l3_maps = []
    for b in range(B):
        m02 = r2[2 * b]["m_out"]
        m13t = t_spatial(r2[2 * b + 1]["m_out"])
        for half in range(2):
            sl = slice(half * LH, (half + 1) * LH)
            l3_maps.append({
                "m02": np.ascontiguousarray(m02[:, sl]),
                "m13": np.ascontiguousarray(m13t[:, sl]),
                "z_in": np.ascontiguousarray(zf[b][:, sl]),
                "lng": ln_g.reshape(Cm, 1),
                "lnb": ln_b.reshape(Cm, 1),
                "wfin": np.ascontiguousarray(W_final.T),
                "bfin": np.ascontiguousarray(cv2_b.reshape(2, 128).T),
                "ones128": np.full((Cm, 1), 1.0 / Cm, np.float32),
            })
    r3 = _run(nc3, l3_maps, "L3")

    out = np.empty((B, C2, H, W), np.float32)
    for core in range(8):
        b, half = core // 2, core % 2
        sl = slice(half * LH, (half + 1) * LH)
        out[b].reshape(C2, L)[:, sl] = r3[core]["d_out"].astype(np.float32)
    out += x
    return out
